# revision 9
# baseline (speedup 1.0000x reference)
"""Trainium2 Bass kernel for an attention-LSTM decoder (scan over 128 steps).

Data-parallel over batch: 64 batches -> 8 cores x 8 batches. All weights and
the per-core encoder slice live SBUF-resident in bf16; the 128-step recurrence
runs in a For_i loop with feature-major (transposed) activation layouts so
every matmul has its contraction dim on partitions and no runtime transposes
are needed (except a single PE transpose of each step's output for DMA).

Self-contained: hardcodes all shapes; imports the Bass/Tile stack from the
machine-wide /opt/trn_rl_repo checkout.
"""
import sys

sys.path.insert(0, "/opt/trn_rl_repo")
import contextlib

import ml_dtypes
import numpy as np

import concourse.bacc as bacc
import concourse.bass as bass
import concourse.tile as tile
from concourse import mybir

B, ENC, DEC, H = 64, 256, 128, 512
IN = 2 * H
NCORES = 8
BL = B // NCORES  # 8 batches per core

F32 = mybir.dt.float32
BF16 = mybir.dt.bfloat16
TANH = mybir.ActivationFunctionType.Tanh
EXP = mybir.ActivationFunctionType.Exp
MULT = mybir.AluOpType.mult
ADD = mybir.AluOpType.add


def build_nc(dec=DEC, unroll=2, loop_mult=1):
    nc = bacc.Bacc("TRN2", num_devices=NCORES, debug=False)

    d_wi0 = nc.dram_tensor("wi0", [128, 8 * 4 * H], BF16, kind="ExternalInput")
    d_wh0 = nc.dram_tensor("wh0", [128, 4 * 4 * H], BF16, kind="ExternalInput")
    d_wi1 = nc.dram_tensor("wi1", [128, 4 * 4 * H], BF16, kind="ExternalInput")
    d_wh1 = nc.dram_tensor("wh1", [128, 4 * 4 * H], BF16, kind="ExternalInput")
    d_wat = nc.dram_tensor("wat", [128, 12 * ENC], BF16, kind="ExternalInput")
    d_wou = nc.dram_tensor("wou", [128, 4 * IN], BF16, kind="ExternalInput")
    d_enc = nc.dram_tensor("enc", [128, BL * 2 * IN], BF16, kind="ExternalInput")
    d_msk = nc.dram_tensor("msk", [128, 16], F32, kind="ExternalInput")
    d_bat = nc.dram_tensor("bat", [128, 2], F32, kind="ExternalInput")
    d_b0 = nc.dram_tensor("b0", [128, 128], F32, kind="ExternalInput")
    d_b1 = nc.dram_tensor("b1", [128, 128], F32, kind="ExternalInput")
    d_bo = nc.dram_tensor("bo", [128, 64], F32, kind="ExternalInput")
    d_id = nc.dram_tensor("id128", [128, 128], F32, kind="ExternalInput")
    # y layout: [b, t, g, p] with output element (b, t, 128*g + p)
    d_y = nc.dram_tensor("y", [BL, dec, 8, 128], F32, kind="ExternalOutput")

    with tile.TileContext(nc) as tc:
        with contextlib.ExitStack() as ctx:
            cpool = ctx.enter_context(tc.tile_pool(name="cpool", bufs=1))
            state = ctx.enter_context(tc.tile_pool(name="state", bufs=1))
            work = ctx.enter_context(tc.tile_pool(name="work", bufs=3))
            psum = ctx.enter_context(tc.tile_pool(name="psum", bufs=1, space="PSUM"))

            # ---- load constants ----
            def load(dram, shape, dtype, nsplit=1, tag=None):
                t = cpool.tile(shape, dtype, tag=tag)
                cols = shape[1]
                step = cols // nsplit
                for i in range(nsplit):
                    nc.gpsimd.dma_start(
                        t[:, i * step : (i + 1) * step],
                        dram[:, i * step : (i + 1) * step],
                    )
                return t

            wi0 = load(d_wi0, [128, 8 * 4 * H], BF16, nsplit=4, tag="wi0")
            wh0 = load(d_wh0, [128, 4 * 4 * H], BF16, nsplit=2, tag="wh0")
            wi1 = load(d_wi1, [128, 4 * 4 * H], BF16, nsplit=2, tag="wi1")
            wh1 = load(d_wh1, [128, 4 * 4 * H], BF16, nsplit=2, tag="wh1")
            wat = load(d_wat, [128, 12 * ENC], BF16, tag="wat")
            wou = load(d_wou, [128, 4 * IN], BF16, tag="wou")
            enc = load(d_enc, [128, BL * 2 * IN], BF16, nsplit=4, tag="enc")
            msk = load(d_msk, [128, 16], F32, tag="msk")
            bat = load(d_bat, [128, 2], F32, tag="bat")
            b0 = load(d_b0, [128, 128], F32, tag="b0")
            b1 = load(d_b1, [128, 128], F32, tag="b1")
            bo = load(d_bo, [128, 64], F32, tag="bo")
            id128 = load(d_id, [128, 128], F32, tag="id128")
            ones_k = cpool.tile([128, 1], F32)
            nc.vector.memset(ones_k, 1.0)
            ones_m = cpool.tile([1, 128], F32)
            nc.vector.memset(ones_m, 1.0)

            # ---- recurrent state (feature-major) ----
            c0 = state.tile([128, 32], F32)
            c1 = state.tile([128, 32], F32)
            h0 = state.tile([128, 32], BF16)
            h1 = state.tile([128, 32], BF16)
            out = state.tile([128, 64], BF16)
            for t in (c0, c1, h0, h1, out):
                nc.vector.memset(t, 0.0)

            def cell(ps_g, bias, cT, hT):
                ga = work.tile([128, 128], F32, tag="ga")
                nc.vector.tensor_add(ga, ps_g, bias)
                ta = work.tile([128, 128], F32, tag="ta")
                nc.scalar.activation(ta[:, 0:64], ga[:, 0:64], TANH, scale=0.5)
                nc.scalar.activation(ta[:, 64:96], ga[:, 64:96], TANH)
                nc.scalar.activation(ta[:, 96:128], ga[:, 96:128], TANH, scale=0.5)
                sif = work.tile([128, 64], F32, tag="sif")
                nc.vector.tensor_scalar(sif, ta[:, 0:64], 0.5, 0.5, MULT, ADD)
                so = work.tile([128, 32], F32, tag="so")
                nc.vector.tensor_scalar(so, ta[:, 96:128], 0.5, 0.5, MULT, ADD)
                v = work.tile([128, 32], F32, tag="v")
                nc.vector.tensor_mul(v, sif[:, 0:32], ta[:, 64:96])
                u = work.tile([128, 32], F32, tag="u")
                nc.vector.tensor_mul(u, sif[:, 32:64], cT)
                nc.vector.tensor_add(cT, u, v)
                tc2 = work.tile([128, 32], F32, tag="tc2")
                nc.scalar.activation(tc2, cT, TANH)
                nc.vector.tensor_mul(hT, so, tc2)

            def step(t_sv):
                # ---- attention scores: psc[p=enc_row%128, mt*8+b] ----
                psc = psum.tile([128, 16], F32, tag="psc")
                order = [8, 9, 10, 11] + list(range(8))  # h0 part first
                for j, kc in enumerate(order):
                    for mt in range(2):
                        if kc >= 8:
                            rhs = h0[:, (kc - 8) * 8 : (kc - 8) * 8 + 8]
                        else:
                            rhs = out[:, kc * 8 : kc * 8 + 8]
                        o = kc * ENC + 128 * mt
                        nc.tensor.matmul(
                            psc[:, mt * 8 : mt * 8 + 8],
                            lhsT=wat[:, o : o + 128],
                            rhs=rhs,
                            start=(j == 0 and mt == 0),
                            stop=(j == 11 and mt == 1),
                        )
                # ---- masked softmax (no max-subtraction; scores are O(5)) ----
                expf = work.tile([128, 16], F32, tag="expf")
                for mt in range(2):
                    nc.scalar.activation(
                        expf[:, mt * 8 : mt * 8 + 8],
                        psc[:, mt * 8 : mt * 8 + 8],
                        EXP,
                        bias=bat[:, mt : mt + 1],
                    )
                expm = work.tile([128, 16], F32, tag="expm")
                nc.vector.tensor_mul(expm, expf, msk)
                ps_s = psum.tile([1, 8], F32, tag="ps_s")
                nc.tensor.matmul(
                    ps_s[0:1, 0:8], lhsT=ones_k, rhs=expm[:, 0:8],
                    start=True, stop=False,
                )
                nc.tensor.matmul(
                    ps_s[0:1, 0:8], lhsT=ones_k, rhs=expm[:, 8:16],
                    start=False, stop=True,
                )
                rec = work.tile([1, 8], F32, tag="rec")
                nc.vector.reciprocal(rec[0:1, 0:8], ps_s[0:1, 0:8])
                ps_r = psum.tile([128, 8], F32, tag="ps_r")
                nc.tensor.matmul(
                    ps_r[:, 0:8], lhsT=ones_m[0:1, 0:128], rhs=rec[0:1, 0:8],
                    start=True, stop=True,
                )
                attnb = work.tile([128, 16], BF16, tag="attnb")
                nc.vector.tensor_mul(attnb[:, 0:8], expm[:, 0:8], ps_r[:, 0:8])
                nc.vector.tensor_mul(attnb[:, 8:16], expm[:, 8:16], ps_r[:, 0:8])

                # ---- ctx[p=d%128, dt*8+b] = sum_e attn[b,e] enc[b,e,d] ----
                ps_ctx = psum.tile([128, 64], F32, tag="ps_ctx")
                for b in range(BL):
                    for dt_ in range(8):
                        for kc in range(2):
                            o = (b * 2 + kc) * IN + 128 * dt_
                            nc.tensor.matmul(
                                ps_ctx[:, dt_ * 8 + b : dt_ * 8 + b + 1],
                                lhsT=enc[:, o : o + 128],
                                rhs=attnb[:, kc * 8 + b : kc * 8 + b + 1],
                                start=(b == 0 and dt_ == 0 and kc == 0),
                                stop=(b == BL - 1 and dt_ == 7 and kc == 1),
                            )
                ctxb = work.tile([128, 64], BF16, tag="ctxb")
                nc.vector.tensor_copy(ctxb, ps_ctx)

                # ---- LSTM0 gates: [p=gate_row%128, M*8+b], M=0..15 ----
                ps_g0 = psum.tile([128, 128], F32, tag="ps_g0")
                order0 = [8, 9, 10, 11] + list(range(8))  # Wh0 (h old) first
                for j, kc in enumerate(order0):
                    for M in range(16):
                        if kc >= 8:
                            lhsT = wh0[:, (kc - 8) * 4 * H + 128 * M :]
                            rhs = h0[:, (kc - 8) * 8 : (kc - 8) * 8 + 8]
                        else:
                            lhsT = wi0[:, kc * 4 * H + 128 * M :]
                            rhs = ctxb[:, kc * 8 : kc * 8 + 8]
                        nc.tensor.matmul(
                            ps_g0[:, M * 8 : M * 8 + 8],
                            lhsT=lhsT[:, 0:128],
                            rhs=rhs,
                            start=(j == 0 and M == 0),
                            stop=(j == 11 and M == 15),
                        )
                cell(ps_g0, b0, c0, h0)

                # ---- LSTM1 gates ----
                ps_g1 = psum.tile([128, 128], F32, tag="ps_g1")
                order1 = [4, 5, 6, 7] + list(range(4))  # Wh1 (h1 old) first
                for j, kc in enumerate(order1):
                    for M in range(16):
                        if kc >= 4:
                            lhsT = wh1[:, (kc - 4) * 4 * H + 128 * M :]
                            rhs = h1[:, (kc - 4) * 8 : (kc - 4) * 8 + 8]
                        else:
                            lhsT = wi1[:, kc * 4 * H + 128 * M :]
                            rhs = h0[:, kc * 8 : kc * 8 + 8]
                        nc.tensor.matmul(
                            ps_g1[:, M * 8 : M * 8 + 8],
                            lhsT=lhsT[:, 0:128],
                            rhs=rhs,
                            start=(j == 0 and M == 0),
                            stop=(j == 7 and M == 15),
                        )
                cell(ps_g1, b1, c1, h1)

                # ---- out = W_out @ h1 + b_out: [p=d%128, g*8+b] ----
                ps_out = psum.tile([128, 64], F32, tag="ps_out")
                for kc in range(4):
                    for g in range(8):
                        o = kc * IN + 128 * g
                        nc.tensor.matmul(
                            ps_out[:, g * 8 : g * 8 + 8],
                            lhsT=wou[:, o : o + 128],
                            rhs=h1[:, kc * 8 : kc * 8 + 8],
                            start=(kc == 0 and g == 0),
                            stop=(kc == 3 and g == 7),
                        )
                outf = work.tile([128, 64], F32, tag="outf")
                nc.vector.tensor_add(outf, ps_out, bo)
                nc.vector.tensor_copy(out, outf)

                # ---- transpose + store ----
                ps_tr = psum.tile([64, 128], F32, tag="ps_tr")
                nc.tensor.transpose(ps_tr, outf, id128)
                otr = work.tile([64, 128], F32, tag="otr")
                nc.vector.tensor_copy(otr, ps_tr)
                # ps_tr partition q = 8*g + b -> iterate DRAM as (g, b, p)
                dview = d_y[:, bass.ds(t_sv, 1), :, :].rearrange(
                    "b t g p -> t g b p"
                )
                nc.gpsimd.dma_start(dview, otr)

            assert dec % unroll == 0
            niter = dec // unroll
            if niter > 1 or loop_mult > 1:
                with tc.For_i(
                    0, niter * loop_mult, hint_engines=(mybir.EngineType.PE,)
                ) as ivr:
                    iv = ivr % niter if loop_mult > 1 else ivr
                    for k in range(unroll):
                        step(iv * unroll + k)
            else:
                for k in range(unroll):
                    step(k)

    nc.compile()
    return nc


def prep_inputs(inputs):
    """Host-side repack of the reference inputs into the kernel layouts."""
    gi = {k: np.asarray(v) for k, v in inputs.items()}
    bf = ml_dtypes.bfloat16

    def kmajor(w, nk):
        # w: [M, K] -> [128, nk*M] with [p, kc*M + m] = w[m, 128*kc + p]
        M, K = w.shape
        assert K == nk * 128
        return np.ascontiguousarray(
            w.T.reshape(nk, 128, M).transpose(1, 0, 2).reshape(128, nk * M)
        )

    shared = {
        "wi0": kmajor(gi["W_ih0"], 8).astype(bf),
        "wh0": kmajor(gi["W_hh0"], 4).astype(bf),
        "wi1": kmajor(gi["W_ih1"], 4).astype(bf),
        "wh1": kmajor(gi["W_hh1"], 4).astype(bf),
        "wat": kmajor(gi["W_attn"], 12).astype(bf),
        "wou": kmajor(gi["W_out"], 4).astype(bf),
        "bat": np.ascontiguousarray(
            gi["b_attn"].reshape(2, 128).T
        ).astype(np.float32),
        "b0": np.repeat(
            (gi["b_ih0"] + gi["b_hh0"]).reshape(16, 128).T, 8, axis=1
        ).astype(np.float32),
        "b1": np.repeat(
            (gi["b_ih1"] + gi["b_hh1"]).reshape(16, 128).T, 8, axis=1
        ).astype(np.float32),
        "bo": np.repeat(gi["b_out"].reshape(8, 128).T, 8, axis=1).astype(
            np.float32
        ),
        "id128": np.eye(128, dtype=np.float32),
    }
    in_maps = []
    for c in range(NCORES):
        e = gi["encoder2_hiddens"][c * BL : (c + 1) * BL]  # [8, 256, 1024]
        enc_t = np.ascontiguousarray(
            e.reshape(BL, 2, 128, IN).transpose(2, 0, 1, 3).reshape(128, BL * 2 * IN)
        ).astype(bf)
        m = gi["x2_mask"][c * BL : (c + 1) * BL]  # [8, 256] int32
        mf = (1 - m).astype(np.float32).T  # [256, 8]
        msk = np.ascontiguousarray(
            mf.reshape(2, 128, BL).transpose(1, 0, 2).reshape(128, 16)
        )
        in_maps.append({**shared, "enc": enc_t, "msk": msk})
    return in_maps


_cache = {}


def _get_nc(dec=DEC, unroll=2, loop_mult=1):
    key = (dec, unroll, loop_mult)
    if key not in _cache:
        _cache[key] = build_nc(dec, unroll, loop_mult)
    return _cache[key]


class Runner:
    """Jit-compiles the Bass program once; repeat calls reuse the executable
    and the device-resident input shards (only fresh output buffers are
    shipped per call when donation is enabled)."""

    def __init__(self, nc, donate=True):
        import jax
        from concourse import bass2jax
        from jax.experimental.shard_map import shard_map
        from jax.sharding import Mesh, PartitionSpec

        bass2jax.install_neuronx_cc_hook()
        self.jax = jax
        self.nc = nc
        self.donate = donate
        pname = nc.partition_id_tensor.name if nc.partition_id_tensor else None
        in_names, out_names, out_avals, zero_outs = [], [], [], []
        for alloc in nc.m.functions[0].allocations:
            if not isinstance(alloc, mybir.MemoryLocationSet):
                continue
            name = alloc.memorylocations[0].name
            if alloc.kind == "ExternalInput":
                if name != pname:
                    in_names.append(name)
            elif alloc.kind == "ExternalOutput":
                shape = tuple(alloc.tensor_shape)
                dtype = mybir.dt.np(alloc.dtype)
                out_names.append(name)
                out_avals.append(jax.core.ShapedArray(shape, dtype))
                zero_outs.append(np.zeros(shape, dtype))
        self.in_names, self.out_names = in_names, out_names
        self.out_avals, self.zero_outs = out_avals, zero_outs
        n_params, n_outs = len(in_names), len(out_names)
        all_names = in_names + out_names + ([pname] if pname else [])

        def _body(*args):
            operands = list(args)
            if pname is not None:
                operands.append(bass2jax.partition_id_tensor())
            outs = bass2jax._bass_exec_p.bind(
                *operands,
                out_avals=tuple(out_avals),
                in_names=tuple(all_names),
                out_names=tuple(out_names),
                lowering_input_output_aliases=(),
                sim_require_finite=True,
                sim_require_nnan=True,
                nc=nc,
            )
            return tuple(outs)

        devices = jax.devices()[:NCORES]
        assert len(devices) == NCORES
        self.mesh = Mesh(np.asarray(devices), ("core",))
        in_specs = (PartitionSpec("core"),) * (n_params + n_outs)
        out_specs = (PartitionSpec("core"),) * n_outs
        kw = dict(donate_argnums=tuple(range(n_params, n_params + n_outs))) if donate else {}
        self.fn = jax.jit(
            shard_map(
                _body, mesh=self.mesh, in_specs=in_specs, out_specs=out_specs,
                check_rep=False,
            ),
            keep_unused=True,
            **kw,
        )
        self._dev_in = None

    def _globalize(self, in_maps):
        jax = self.jax
        from jax.sharding import NamedSharding, PartitionSpec

        sh = NamedSharding(self.mesh, PartitionSpec("core"))
        arrs = []
        for name in self.in_names:
            g = np.concatenate([np.asarray(m[name]) for m in in_maps], axis=0)
            arrs.append(jax.device_put(g, sh))
        return arrs

    def set_inputs(self, in_maps):
        self._dev_in = self._globalize(in_maps)

    def _zeros_dev(self):
        from jax.sharding import NamedSharding, PartitionSpec

        sh = NamedSharding(self.mesh, PartitionSpec("core"))
        return [
            self.jax.device_put(
                np.zeros((NCORES * z.shape[0], *z.shape[1:]), z.dtype), sh
            )
            for z in self.zero_outs
        ]

    def __call__(self):
        outs = self.fn(*self._dev_in, *self._zeros_dev())
        return outs

    def gather(self, outs):
        res = []
        for i, name in enumerate(self.out_names):
            a = np.asarray(outs[i])
            res.append(a.reshape(NCORES, *self.out_avals[i].shape))
        return dict(zip(self.out_names, res))


_runner_cache = {}


def get_runner(dec=DEC, unroll=2, donate=True, loop_mult=1):
    key = (dec, unroll, donate, loop_mult)
    if key not in _runner_cache:
        _runner_cache[key] = Runner(
            _get_nc(dec, unroll, loop_mult), donate=donate
        )
    return _runner_cache[key]


def run_on_hw(inputs, dec=DEC, unroll=2):
    r = get_runner(dec, unroll)
    r.set_inputs(prep_inputs(inputs))
    outs = r()
    y = r.gather(outs)["y"]  # [NCORES, BL, dec, 8, 128]
    return y.reshape(B, dec, IN).astype(np.float32)


def kernel(**inputs):
    return run_on_hw(inputs)
